# revision 23
# baseline (speedup 1.0000x reference)
"""ClusterKLLoss Trainium2 kernel — 8 NeuronCores, j-stripe data-parallel.

Math (from the reference):
  loss·B = sum_i lse_j(G[i,j]) - sum_i G[i,i]
  G[i,j] = (c_i[i]·Q_j - hneg_j)/T,  Q_j = E_j/Z_j,  E = exp(c_j),
  Z_j = sum_k E[j,k],  A_j = sum_k E[j,k]·c_j[j,k],  hneg_j = A_j/Z_j - ln Z_j.

Sharding: core c owns j-stripe [512c, 512c+512). It computes E, Z, A for its
stripe, forms W^T = E^T in fp8, and computes S^T[j,i] against the full
transposed c_i (fp8 input, moving operand) with fp8 DoubleRow matmuls (2x PE
rate, stationary = local E^T tiles). The per-j softmax scale s_j = 1/(T·Z_j)
and bias b_j = lnZ_j/T - A_j·s_j ride the Exp activation's per-partition
scale/bias (S^T has j on partitions). exp tiles accumulate over j-tiles into
Texp[i] (vector adds), are column-summed via one-hot-stationary matmuls, and
an AllReduce(add) across the 8 cores combines per-i partial sums + diagonal
partials; every core computes the identical final scalar on device.

The diagonal G_ii (i in own stripe) is computed position-independently from
natural-layout fp16 tiles via a fused multiply-reduce:
diag_S[i] = sum_k cid[i,k]·E[i,k], G_ii = diag_S·s_i + b_i.

Emission is software-pipelined (P0 P1 M0 P2 M1 P3 M2 M3); ciT lives in 8
per-chunk tiles whose loads are all issued up front across the sync, gpsimd
and scalar DMA queues, so matmuls gate only on their own chunk.
"""

import os
import sys

for _p in ("/opt/trn_rl_repo",):
    if _p not in sys.path:
        sys.path.insert(0, _p)

import numpy as np
import ml_dtypes

import concourse.bass as bass
import concourse.bacc as bacc
import concourse.tile as tile
from concourse import mybir
from concourse import bass_utils

B = 4096
D = 2048
TEMP = 0.5
NCORES = 8
SHARD = B // NCORES  # 512
KT = D // 128  # 16 k partition-tiles
KP = KT // 2  # 8 DoubleRow k-pairs
JT = SHARD // 128  # 4 j-tiles per stripe
ICH = B // 512  # 8 i-chunks of 512

F32 = mybir.dt.float32
F16 = mybir.dt.float16
BF16 = mybir.dt.bfloat16
F8 = mybir.dt.float8e4
AF = mybir.ActivationFunctionType
OP = mybir.AluOpType
AX = mybir.AxisListType
DR = mybir.MatmulPerfMode.DoubleRow

DEBUG_OUT = os.environ.get("K_DEBUG_OUT", "0") == "1"
HOST_COMBINE = os.environ.get("K_HOST_COMBINE", "0") == "1"
USE_TTR = os.environ.get("K_TTR", "0") == "1"
USE_MIXADD = os.environ.get("K_MIXADD", "0") == "1"
USE_STT = os.environ.get("K_STT", "1") == "1"


def build_kernel_body(tc, out_ap, cit8_ap, cj16_ap, cjt16_ap, cid16_ap, oh_ap,
                      dbg=None, host_combine=False):
    nc = tc.nc
    from contextlib import ExitStack

    with ExitStack() as ctx:
        singles = ctx.enter_context(tc.tile_pool(name="singles", bufs=1))
        xpool = ctx.enter_context(tc.tile_pool(name="xpool", bufs=8))
        epool = ctx.enter_context(tc.tile_pool(name="epool", bufs=2))
        ppool = ctx.enter_context(tc.tile_pool(name="ppool", bufs=1))
        etpool = ctx.enter_context(tc.tile_pool(name="etpool", bufs=2))
        e32pool = ctx.enter_context(tc.tile_pool(name="e32pool", bufs=3))
        spool = ctx.enter_context(tc.tile_pool(name="spool", bufs=4))
        psS = ctx.enter_context(tc.tile_pool(name="psS", bufs=6, space="PSUM"))
        psE = ctx.enter_context(tc.tile_pool(name="psE", bufs=1, space="PSUM"))
        dram = ctx.enter_context(tc.tile_pool(name="dram", bufs=1, space="DRAM"))

        # All stripe loads go first on the gpsimd queue (the critical exp ->
        # transpose -> fp8 chain hangs off xt0), then the last two ciT chunks.
        # ciT chunks 0-5 stream on the sync queue; the scalar queue carries
        # only the transposes so they never wait behind megabyte loads.
        xts = [None] * JT
        cjts = [None] * JT
        cdts = [None] * JT
        for jt in range(JT):
            xt = xpool.tile([128, D], F16, tag="xt", name=f"xt{jt}")
            nc.gpsimd.dma_start(out=xt, in_=cj16_ap[128 * jt : 128 * (jt + 1), :])
            cjt = xpool.tile([128, KT, 128], F16, tag="cjt", name=f"cjt{jt}", bufs=4)
            nc.gpsimd.dma_start(
                out=cjt, in_=cjt16_ap[:, :, 128 * jt : 128 * (jt + 1)]
            )
            cdt = xpool.tile([128, D], F16, tag="xt", name=f"cdt{jt}")
            nc.gpsimd.dma_start(out=cdt, in_=cid16_ap[128 * jt : 128 * (jt + 1), :])
            xts[jt] = xt
            cjts[jt] = cjt
            cdts[jt] = cdt

        ciTs = [
            singles.tile([128, KT, 512], F8, name=f"ciT{c}") for c in range(ICH)
        ]
        for c in range(4):
            nc.sync.dma_start(
                out=ciTs[c], in_=cit8_ap[:, :, 512 * c : 512 * (c + 1)]
            )
        for c in (4, 5, 6):
            nc.scalar.dma_start(
                out=ciTs[c], in_=cit8_ap[:, :, 512 * c : 512 * (c + 1)]
            )
        nc.gpsimd.dma_start(out=ciTs[7], in_=cit8_ap[:, :, 512 * 7 : 512 * 8])

        oh = singles.tile([128, 8, 8], BF16)
        nc.sync.dma_start(out=oh, in_=oh_ap)
        onesf = singles.tile([128, 1], F32)
        nc.vector.memset(onesf, 1.0)
        ones8 = singles.tile([8, 1], F32)
        nc.vector.memset(ones8, 1.0)

        Z = singles.tile([128, JT], F32)
        A = singles.tile([128, JT], F32)
        sj = singles.tile([128, JT], F32)
        bneg = singles.tile([128, JT], F32)
        zsq = singles.tile([128, JT], F32)
        draw = singles.tile([128, JT], F32)
        Texp = singles.tile([128, B], F32)
        Texbf = singles.tile([128, B], BF16)
        ET8s = [None] * JT

        def phase_p(jt):
            xt = xts[jt]
            cdt = cdts[jt]
            es = epool.tile([128, D], F16, tag="es")
            nc.scalar.activation(
                out=es, in_=xt, func=AF.Exp, accum_out=Z[:, jt : jt + 1]
            )
            # W^T = E^T (fp8) straight from the transposed input: no DMA
            # transpose, exp works elementwise in any layout.
            ET16 = etpool.tile([128, KT, 128], F16, tag="et16")
            nc.scalar.activation(out=ET16, in_=cjts[jt], func=AF.Exp)
            ET8 = etpool.tile([128, KT, 128], F8, tag="et8")
            nc.vector.tensor_copy(out=ET8, in_=ET16)
            ET8s[jt] = ET8
            # s_j = 1/(T*Z); zsq = Z^2 (exp(G) = exp((M-A_j)s_j) * Z_j^2, so
            # no Ln and no activation-table switch in the main pipeline)
            nc.vector.tensor_scalar_mul(sj[:, jt : jt + 1], Z[:, jt : jt + 1], TEMP)
            nc.vector.reciprocal(out=sj[:, jt : jt + 1], in_=sj[:, jt : jt + 1])
            nc.vector.tensor_mul(
                zsq[:, jt : jt + 1], Z[:, jt : jt + 1], Z[:, jt : jt + 1]
            )
            # A = sum E*x and diag raw = sum cid*E
            prod = ppool.tile([128, D], F16, tag="prod")
            if USE_STT:
                nc.vector.scalar_tensor_tensor(
                    out=prod, in0=es, scalar=1.0, in1=xt,
                    op0=OP.mult, op1=OP.mult, accum_out=A[:, jt : jt + 1],
                )
            else:
                nc.vector.tensor_mul(prod, es, xt)
                nc.vector.tensor_reduce(
                    out=A[:, jt : jt + 1], in_=prod, axis=AX.X, op=OP.add
                )
            tmp = spool.tile([128, 1], F32, tag="tmp")
            nc.vector.tensor_mul(tmp, A[:, jt : jt + 1], sj[:, jt : jt + 1])
            nc.vector.tensor_scalar_mul(bneg[:, jt : jt + 1], tmp, -1.0)
            dprod = ppool.tile([128, D], F16, tag="dprod")
            if USE_STT:
                nc.vector.scalar_tensor_tensor(
                    out=dprod, in0=cdt, scalar=1.0, in1=es,
                    op0=OP.mult, op1=OP.mult, accum_out=draw[:, jt : jt + 1],
                )
            else:
                nc.vector.tensor_mul(dprod, cdt, es)
                nc.vector.tensor_reduce(
                    out=draw[:, jt : jt + 1], in_=dprod, axis=AX.X, op=OP.add
                )

        def phase_m(jt):
            # S^T[j, i] over full i, fp8 DoubleRow, in 2 psum half-sweeps
            ET8 = ET8s[jt]
            for h in range(2):
                S = [
                    psS.tile([128, 512], F32, tag="s", name=f"S{jt}_{h}_{q}")
                    for q in range(4)
                ]
                for kp in range(KP):
                    for q in range(4):
                        c = 4 * h + q
                        nc.tensor.matmul(
                            S[q],
                            ET8[:, 2 * kp : 2 * kp + 2, :],
                            ciTs[c][:, 2 * kp : 2 * kp + 2, :],
                            start=(kp == 0),
                            stop=(kp == KP - 1),
                            perf_mode=DR,
                        )
                for q in range(4):
                    c = 4 * h + q
                    sl = slice(512 * c, 512 * (c + 1))
                    e32 = e32pool.tile([128, 512], F32, tag="e32")
                    nc.scalar.activation(
                        out=e32, in_=S[q], func=AF.Exp,
                        scale=sj[:, jt : jt + 1], bias=bneg[:, jt : jt + 1],
                    )
                    if jt == 0:
                        nc.vector.tensor_scalar_mul(
                            Texp[:, sl], e32, zsq[:, jt : jt + 1]
                        )
                    else:
                        if USE_STT:
                            nc.vector.scalar_tensor_tensor(
                                out=Texp[:, sl], in0=e32,
                                scalar=zsq[:, jt : jt + 1], in1=Texp[:, sl],
                                op0=OP.mult, op1=OP.add,
                            )
                        else:
                            nc.vector.tensor_scalar_mul(
                                e32, e32, zsq[:, jt : jt + 1]
                            )
                            nc.vector.tensor_add(Texp[:, sl], Texp[:, sl], e32)
                        if jt == JT - 1:  # bf16 colsum operand, per chunk
                            nc.vector.tensor_copy(
                                out=Texbf[:, sl], in_=Texp[:, sl]
                            )

        # software pipeline: P0 P1 M0 P2 M1 P3 M2 M3
        phase_p(0)
        phase_p(1)
        phase_m(0)
        phase_p(2)
        phase_m(1)
        phase_p(3)
        phase_m(2)
        phase_m(3)

        # diagonal: G_ii = (draw_i - A_i)*s_i + 2*lnZ_i summed over own stripe
        # (the Ln here is the first activation-table switch since the prologue)
        lnzA = spool.tile([128, JT], F32, tag="lnzA")
        nc.scalar.activation(out=lnzA, in_=Z, func=AF.Ln)
        gd = spool.tile([128, JT], F32, tag="gd")
        nc.vector.tensor_sub(gd, draw, A)
        nc.vector.tensor_mul(gd, gd, sj)
        gl = spool.tile([128, JT], F32, tag="gl")
        nc.vector.tensor_scalar_mul(gl, lnzA, 2.0)
        nc.vector.tensor_add(gd, gd, gl)
        dsum = spool.tile([128, 1], F32, tag="dsum")
        nc.vector.tensor_reduce(out=dsum, in_=gd, axis=AX.X, op=OP.add)
        dps = psE.tile([1, 1], F32, tag="e")
        nc.tensor.matmul(dps, onesf, dsum, start=True, stop=True)
        dsc = spool.tile([1, 1], F32, tag="dsc")
        nc.vector.tensor_copy(out=dsc, in_=dps)

        if dbg is not None:
            zab = spool.tile([128, 5 * JT], F32, tag="zab")
            nc.vector.tensor_copy(out=zab[:, 0:JT], in_=Z)
            nc.vector.tensor_copy(out=zab[:, JT : 2 * JT], in_=A)
            nc.vector.tensor_copy(out=zab[:, 2 * JT : 3 * JT], in_=sj)
            nc.vector.tensor_copy(out=zab[:, 3 * JT : 4 * JT], in_=bneg)
            nc.vector.tensor_copy(out=zab[:, 4 * JT : 5 * JT], in_=draw)
            nc.sync.dma_start(out=dbg["zab"], in_=zab)
            nc.sync.dma_start(out=dbg["texp"], in_=Texp)

        # column sums of Texbf via one-hot stationaries -> [8, 512]
        cs = psE.tile([8, 512], F32, tag="e")
        for r in range(8):
            nc.tensor.matmul(
                cs,
                oh[:, r],
                Texbf[:, 512 * r : 512 * (r + 1)],
                start=(r == 0),
                stop=(r == 7),
            )
        cssb = spool.tile([8, 513], F32, tag="cssb")
        nc.vector.memset(cssb, 0.0)
        nc.vector.tensor_copy(out=cssb[:, 0:512], in_=cs)
        nc.vector.tensor_copy(out=cssb[0:1, 512:513], in_=dsc)

        if host_combine:
            nc.sync.dma_start(out=out_ap, in_=cssb)
            return

        # AllReduce partial rowsums + diag partial across the 8 cores
        bin_ = dram.tile([8, 513], F32)
        bout = dram.tile([8, 513], F32)
        nc.gpsimd.dma_start(bin_, cssb)
        nc.gpsimd.collective_compute(
            "AllReduce",
            OP.add,
            replica_groups=[list(range(NCORES))],
            ins=[bin_.opt()],
            outs=[bout.opt()],
        )
        arsb = spool.tile([8, 513], F32, tag="arsb")
        nc.gpsimd.dma_start(arsb, bout)
        if dbg is not None:
            nc.sync.dma_start(out=dbg["cssb"], in_=cssb)
            nc.sync.dma_start(out=dbg["arsb"], in_=arsb)

        # loss*B = sum_i ln(T_i) - sum_i G_ii  (identical on every core)
        lnt = spool.tile([8, 512], F32, tag="lnt")
        lnacc = spool.tile([8, 1], F32, tag="lnacc")
        nc.scalar.activation(
            out=lnt, in_=arsb[:, 0:512], func=AF.Ln, accum_out=lnacc
        )
        tps = psE.tile([1, 1], F32, tag="e")
        nc.tensor.matmul(tps, ones8, lnacc, start=True, stop=True)
        tsb = spool.tile([1, 1], F32, tag="tsb")
        nc.vector.tensor_copy(out=tsb, in_=tps)
        res = spool.tile([1, 1], F32, tag="res")
        nc.vector.tensor_sub(res, tsb, arsb[0:1, 512:513])
        nc.sync.dma_start(out=out_ap, in_=res)


_NC_CACHE = {}


def build_nc():
    key = ("nc", HOST_COMBINE)
    if key in _NC_CACHE:
        return _NC_CACHE[key]
    nc = bacc.Bacc(
        "TRN2", target_bir_lowering=False, debug=False, num_devices=NCORES
    )
    cit8 = nc.dram_tensor("cit8", [128, KT, B], F8, kind="ExternalInput").ap()
    cj16 = nc.dram_tensor("cj16", [SHARD, D], F16, kind="ExternalInput").ap()
    cjt16 = nc.dram_tensor("cjt16", [128, KT, SHARD], F16, kind="ExternalInput").ap()
    cid16 = nc.dram_tensor("cid16", [SHARD, D], F16, kind="ExternalInput").ap()
    oh = nc.dram_tensor("oh", [128, 8, 8], BF16, kind="ExternalInput").ap()
    out_shape = [8, 513] if HOST_COMBINE else [1, 1]
    out = nc.dram_tensor("out", out_shape, F32, kind="ExternalOutput").ap()
    dbg = None
    if DEBUG_OUT:
        dbg = {
            "zab": nc.dram_tensor("d_zab", [128, 5 * JT], F32, kind="ExternalOutput").ap(),
            "texp": nc.dram_tensor("d_texp", [128, B], F32, kind="ExternalOutput").ap(),
            "cssb": nc.dram_tensor("d_cssb", [8, 513], F32, kind="ExternalOutput").ap(),
            "arsb": nc.dram_tensor("d_arsb", [8, 513], F32, kind="ExternalOutput").ap(),
        }
    with tile.TileContext(nc) as tc:
        build_kernel_body(tc, out, cit8, cj16, cjt16, cid16, oh, dbg=dbg,
                          host_combine=HOST_COMBINE)
    nc.compile()
    _NC_CACHE[key] = nc
    return nc


def make_in_maps(c_i, c_j):
    c_i = np.ascontiguousarray(np.asarray(c_i, dtype=np.float32))
    c_j = np.ascontiguousarray(np.asarray(c_j, dtype=np.float32))
    ci8 = c_i.astype(ml_dtypes.float8_e4m3)
    # cit8[p, kt, i] = c_i[i, kt*128 + p]
    cit8 = np.ascontiguousarray(ci8.T.reshape(KT, 128, B).transpose(1, 0, 2))
    ci16 = c_i.astype(np.float16)
    cj16 = c_j.astype(np.float16)
    oh = np.zeros((128, 8, 8), ml_dtypes.bfloat16)
    for r in range(8):
        oh[:, r, r] = 1.0
    in_maps = []
    for c in range(NCORES):
        cjs = cj16[SHARD * c : SHARD * (c + 1)]
        # cjt16[p, kt, j] = c_j[512c + j, kt*128 + p]
        cjt = np.ascontiguousarray(
            cjs.T.reshape(KT, 128, SHARD).transpose(1, 0, 2)
        )
        in_maps.append(
            {
                "cit8": cit8,
                "cj16": np.ascontiguousarray(cjs),
                "cjt16": cjt,
                "cid16": np.ascontiguousarray(ci16[SHARD * c : SHARD * (c + 1)]),
                "oh": oh,
            }
        )
    return in_maps


def kernel(c_i, c_j, **kwargs):
    nc = build_nc()
    in_maps = make_in_maps(c_i, c_j)
    res = bass_utils.run_bass_kernel_spmd(
        nc, in_maps, core_ids=list(range(NCORES))
    )
    if HOST_COMBINE:
        acc = np.zeros((8, 513), np.float64)
        for r in res.results:
            acc += r["out"].astype(np.float64)
        lossB = np.log(acc[:, 0:512]).sum() - acc[0, 512]
        return np.float32(lossB / B).reshape(())
    return np.float32(np.float64(res.results[0]["out"][0, 0]) / B).reshape(())


# revision 24
# speedup vs baseline: 1.0365x; 1.0365x over previous
"""ClusterKLLoss Trainium2 kernel — 8 NeuronCores, j-stripe data-parallel.

Math (from the reference):
  loss·B = sum_i lse_j(G[i,j]) - sum_i G[i,i]
  G[i,j] = (c_i[i]·Q_j - hneg_j)/T,  Q_j = E_j/Z_j,  E = exp(c_j),
  Z_j = sum_k E[j,k],  A_j = sum_k E[j,k]·c_j[j,k],  hneg_j = A_j/Z_j - ln Z_j.

Sharding: core c owns j-stripe [512c, 512c+512). It computes E, Z, A for its
stripe, forms W^T = E^T in fp8, and computes S^T[j,i] against the full
transposed c_i (fp8 input, moving operand) with fp8 DoubleRow matmuls (2x PE
rate, stationary = local E^T tiles). The per-j softmax scale s_j = 1/(T·Z_j)
and bias b_j = lnZ_j/T - A_j·s_j ride the Exp activation's per-partition
scale/bias (S^T has j on partitions). exp tiles accumulate over j-tiles into
Texp[i] (vector adds), are column-summed via one-hot-stationary matmuls, and
an AllReduce(add) across the 8 cores combines per-i partial sums + diagonal
partials; every core computes the identical final scalar on device.

The diagonal G_ii (i in own stripe) is computed position-independently from
natural-layout fp16 tiles via a fused multiply-reduce:
diag_S[i] = sum_k cid[i,k]·E[i,k], G_ii = diag_S·s_i + b_i.

Emission is software-pipelined (P0 P1 M0 P2 M1 P3 M2 M3); ciT lives in 8
per-chunk tiles whose loads are all issued up front across the sync, gpsimd
and scalar DMA queues, so matmuls gate only on their own chunk.
"""

import os
import sys

for _p in ("/opt/trn_rl_repo",):
    if _p not in sys.path:
        sys.path.insert(0, _p)

import numpy as np
import ml_dtypes

import concourse.bass as bass
import concourse.bacc as bacc
import concourse.tile as tile
from concourse import mybir
from concourse import bass_utils

B = 4096
D = 2048
TEMP = 0.5
NCORES = 8
SHARD = B // NCORES  # 512
KT = D // 128  # 16 k partition-tiles
KP = KT // 2  # 8 DoubleRow k-pairs
JT = SHARD // 128  # 4 j-tiles per stripe
ICH = B // 512  # 8 i-chunks of 512

F32 = mybir.dt.float32
F16 = mybir.dt.float16
BF16 = mybir.dt.bfloat16
F8 = mybir.dt.float8e4
AF = mybir.ActivationFunctionType
OP = mybir.AluOpType
AX = mybir.AxisListType
DR = mybir.MatmulPerfMode.DoubleRow

DEBUG_OUT = os.environ.get("K_DEBUG_OUT", "0") == "1"
HOST_COMBINE = os.environ.get("K_HOST_COMBINE", "0") == "1"
USE_TTR = os.environ.get("K_TTR", "0") == "1"
USE_MIXADD = os.environ.get("K_MIXADD", "0") == "1"
USE_STT = os.environ.get("K_STT", "1") == "1"


def build_kernel_body(tc, out_ap, cit8_ap, cj16_ap, cjt16_ap, cid16_ap, oh_ap,
                      dbg=None, host_combine=False):
    nc = tc.nc
    from contextlib import ExitStack

    with ExitStack() as ctx:
        singles = ctx.enter_context(tc.tile_pool(name="singles", bufs=1))
        xpool = ctx.enter_context(tc.tile_pool(name="xpool", bufs=8))
        epool = ctx.enter_context(tc.tile_pool(name="epool", bufs=2))
        ppool = ctx.enter_context(tc.tile_pool(name="ppool", bufs=1))
        etpool = ctx.enter_context(tc.tile_pool(name="etpool", bufs=2))
        e32pool = ctx.enter_context(tc.tile_pool(name="e32pool", bufs=3))
        spool = ctx.enter_context(tc.tile_pool(name="spool", bufs=4))
        psS = ctx.enter_context(tc.tile_pool(name="psS", bufs=6, space="PSUM"))
        psE = ctx.enter_context(tc.tile_pool(name="psE", bufs=1, space="PSUM"))
        dram = ctx.enter_context(tc.tile_pool(name="dram", bufs=1, space="DRAM"))

        # All stripe loads go first on the gpsimd queue (the critical exp ->
        # transpose -> fp8 chain hangs off xt0), then the last two ciT chunks.
        # ciT chunks 0-5 stream on the sync queue; the scalar queue carries
        # only the transposes so they never wait behind megabyte loads.
        xts = [None] * JT
        cjts = [None] * JT
        cdts = [None] * JT
        for jt in range(JT):
            xt = xpool.tile([128, D], F16, tag="xt", name=f"xt{jt}")
            nc.gpsimd.dma_start(out=xt, in_=cj16_ap[128 * jt : 128 * (jt + 1), :])
            cjt = xpool.tile([128, KT, 128], F16, tag="cjt", name=f"cjt{jt}", bufs=4)
            nc.gpsimd.dma_start(out=cjt, in_=cjt16_ap[jt])
            cdt = xpool.tile([128, D], F16, tag="xt", name=f"cdt{jt}")
            nc.gpsimd.dma_start(out=cdt, in_=cid16_ap[128 * jt : 128 * (jt + 1), :])
            xts[jt] = xt
            cjts[jt] = cjt
            cdts[jt] = cdt

        ciTs = [
            singles.tile([128, KT, 512], F8, name=f"ciT{c}") for c in range(ICH)
        ]
        for c in range(4):
            nc.sync.dma_start(out=ciTs[c], in_=cit8_ap[c])
        for c in (4, 5, 6):
            nc.scalar.dma_start(out=ciTs[c], in_=cit8_ap[c])
        nc.gpsimd.dma_start(out=ciTs[7], in_=cit8_ap[7])

        oh = singles.tile([128, 8, 8], BF16)
        nc.sync.dma_start(out=oh, in_=oh_ap)
        onesf = singles.tile([128, 1], F32)
        nc.vector.memset(onesf, 1.0)
        ones8 = singles.tile([8, 1], F32)
        nc.vector.memset(ones8, 1.0)

        Z = singles.tile([128, JT], F32)
        A = singles.tile([128, JT], F32)
        sj = singles.tile([128, JT], F32)
        bneg = singles.tile([128, JT], F32)
        zsq = singles.tile([128, JT], F32)
        draw = singles.tile([128, JT], F32)
        Texp = singles.tile([128, B], F32)
        Texbf = singles.tile([128, B], BF16)
        ET8s = [None] * JT

        def phase_p(jt):
            xt = xts[jt]
            cdt = cdts[jt]
            es = epool.tile([128, D], F16, tag="es")
            nc.scalar.activation(
                out=es, in_=xt, func=AF.Exp, accum_out=Z[:, jt : jt + 1]
            )
            # W^T = E^T (fp8) straight from the transposed input: no DMA
            # transpose, exp works elementwise in any layout.
            ET16 = etpool.tile([128, KT, 128], F16, tag="et16")
            nc.scalar.activation(out=ET16, in_=cjts[jt], func=AF.Exp)
            ET8 = etpool.tile([128, KT, 128], F8, tag="et8")
            nc.vector.tensor_copy(out=ET8, in_=ET16)
            ET8s[jt] = ET8
            # s_j = 1/(T*Z); zsq = Z^2 (exp(G) = exp((M-A_j)s_j) * Z_j^2, so
            # no Ln and no activation-table switch in the main pipeline)
            nc.vector.tensor_scalar_mul(sj[:, jt : jt + 1], Z[:, jt : jt + 1], TEMP)
            nc.vector.reciprocal(out=sj[:, jt : jt + 1], in_=sj[:, jt : jt + 1])
            nc.vector.tensor_mul(
                zsq[:, jt : jt + 1], Z[:, jt : jt + 1], Z[:, jt : jt + 1]
            )
            # A = sum E*x and diag raw = sum cid*E
            prod = ppool.tile([128, D], F16, tag="prod")
            if USE_STT:
                nc.vector.scalar_tensor_tensor(
                    out=prod, in0=es, scalar=1.0, in1=xt,
                    op0=OP.mult, op1=OP.mult, accum_out=A[:, jt : jt + 1],
                )
            else:
                nc.vector.tensor_mul(prod, es, xt)
                nc.vector.tensor_reduce(
                    out=A[:, jt : jt + 1], in_=prod, axis=AX.X, op=OP.add
                )
            tmp = spool.tile([128, 1], F32, tag="tmp")
            nc.vector.tensor_mul(tmp, A[:, jt : jt + 1], sj[:, jt : jt + 1])
            nc.vector.tensor_scalar_mul(bneg[:, jt : jt + 1], tmp, -1.0)
            dprod = ppool.tile([128, D], F16, tag="dprod")
            if USE_STT:
                nc.vector.scalar_tensor_tensor(
                    out=dprod, in0=cdt, scalar=1.0, in1=es,
                    op0=OP.mult, op1=OP.mult, accum_out=draw[:, jt : jt + 1],
                )
            else:
                nc.vector.tensor_mul(dprod, cdt, es)
                nc.vector.tensor_reduce(
                    out=draw[:, jt : jt + 1], in_=dprod, axis=AX.X, op=OP.add
                )

        def phase_m(jt):
            # S^T[j, i] over full i, fp8 DoubleRow, in 2 psum half-sweeps
            ET8 = ET8s[jt]
            for h in range(2):
                S = [
                    psS.tile([128, 512], F32, tag="s", name=f"S{jt}_{h}_{q}")
                    for q in range(4)
                ]
                for kp in range(KP):
                    for q in range(4):
                        c = 4 * h + q
                        nc.tensor.matmul(
                            S[q],
                            ET8[:, 2 * kp : 2 * kp + 2, :],
                            ciTs[c][:, 2 * kp : 2 * kp + 2, :],
                            start=(kp == 0),
                            stop=(kp == KP - 1),
                            perf_mode=DR,
                        )
                for q in range(4):
                    c = 4 * h + q
                    sl = slice(512 * c, 512 * (c + 1))
                    e32 = e32pool.tile([128, 512], F32, tag="e32")
                    nc.scalar.activation(
                        out=e32, in_=S[q], func=AF.Exp,
                        scale=sj[:, jt : jt + 1], bias=bneg[:, jt : jt + 1],
                    )
                    if jt == 0:
                        nc.vector.tensor_scalar_mul(
                            Texp[:, sl], e32, zsq[:, jt : jt + 1]
                        )
                    else:
                        if USE_STT:
                            nc.vector.scalar_tensor_tensor(
                                out=Texp[:, sl], in0=e32,
                                scalar=zsq[:, jt : jt + 1], in1=Texp[:, sl],
                                op0=OP.mult, op1=OP.add,
                            )
                        else:
                            nc.vector.tensor_scalar_mul(
                                e32, e32, zsq[:, jt : jt + 1]
                            )
                            nc.vector.tensor_add(Texp[:, sl], Texp[:, sl], e32)
                        if jt == JT - 1:  # bf16 colsum operand, per chunk
                            nc.vector.tensor_copy(
                                out=Texbf[:, sl], in_=Texp[:, sl]
                            )

        # software pipeline: P0 P1 M0 P2 M1 P3 M2 M3
        phase_p(0)
        phase_p(1)
        phase_m(0)
        phase_p(2)
        phase_m(1)
        phase_p(3)
        phase_m(2)
        phase_m(3)

        # diagonal: G_ii = (draw_i - A_i)*s_i + 2*lnZ_i summed over own stripe
        # (the Ln here is the first activation-table switch since the prologue)
        lnzA = spool.tile([128, JT], F32, tag="lnzA")
        nc.scalar.activation(out=lnzA, in_=Z, func=AF.Ln)
        gd = spool.tile([128, JT], F32, tag="gd")
        nc.vector.tensor_sub(gd, draw, A)
        nc.vector.tensor_mul(gd, gd, sj)
        gl = spool.tile([128, JT], F32, tag="gl")
        nc.vector.tensor_scalar_mul(gl, lnzA, 2.0)
        nc.vector.tensor_add(gd, gd, gl)
        dsum = spool.tile([128, 1], F32, tag="dsum")
        nc.vector.tensor_reduce(out=dsum, in_=gd, axis=AX.X, op=OP.add)
        dps = psE.tile([1, 1], F32, tag="e")
        nc.tensor.matmul(dps, onesf, dsum, start=True, stop=True)
        dsc = spool.tile([1, 1], F32, tag="dsc")
        nc.vector.tensor_copy(out=dsc, in_=dps)

        if dbg is not None:
            zab = spool.tile([128, 5 * JT], F32, tag="zab")
            nc.vector.tensor_copy(out=zab[:, 0:JT], in_=Z)
            nc.vector.tensor_copy(out=zab[:, JT : 2 * JT], in_=A)
            nc.vector.tensor_copy(out=zab[:, 2 * JT : 3 * JT], in_=sj)
            nc.vector.tensor_copy(out=zab[:, 3 * JT : 4 * JT], in_=bneg)
            nc.vector.tensor_copy(out=zab[:, 4 * JT : 5 * JT], in_=draw)
            nc.sync.dma_start(out=dbg["zab"], in_=zab)
            nc.sync.dma_start(out=dbg["texp"], in_=Texp)

        # column sums of Texbf via one-hot stationaries -> [8, 512]
        cs = psE.tile([8, 512], F32, tag="e")
        for r in range(8):
            nc.tensor.matmul(
                cs,
                oh[:, r],
                Texbf[:, 512 * r : 512 * (r + 1)],
                start=(r == 0),
                stop=(r == 7),
            )
        cssb = spool.tile([8, 513], F32, tag="cssb")
        nc.vector.memset(cssb, 0.0)
        nc.vector.tensor_copy(out=cssb[:, 0:512], in_=cs)
        nc.vector.tensor_copy(out=cssb[0:1, 512:513], in_=dsc)

        if host_combine:
            nc.sync.dma_start(out=out_ap, in_=cssb)
            return

        # AllReduce partial rowsums + diag partial across the 8 cores
        bin_ = dram.tile([8, 513], F32)
        bout = dram.tile([8, 513], F32)
        nc.gpsimd.dma_start(bin_, cssb)
        nc.gpsimd.collective_compute(
            "AllReduce",
            OP.add,
            replica_groups=[list(range(NCORES))],
            ins=[bin_.opt()],
            outs=[bout.opt()],
        )
        arsb = spool.tile([8, 513], F32, tag="arsb")
        nc.gpsimd.dma_start(arsb, bout)
        if dbg is not None:
            nc.sync.dma_start(out=dbg["cssb"], in_=cssb)
            nc.sync.dma_start(out=dbg["arsb"], in_=arsb)

        # loss*B = sum_i ln(T_i) - sum_i G_ii  (identical on every core)
        lnt = spool.tile([8, 512], F32, tag="lnt")
        lnacc = spool.tile([8, 1], F32, tag="lnacc")
        nc.scalar.activation(
            out=lnt, in_=arsb[:, 0:512], func=AF.Ln, accum_out=lnacc
        )
        tps = psE.tile([1, 1], F32, tag="e")
        nc.tensor.matmul(tps, ones8, lnacc, start=True, stop=True)
        tsb = spool.tile([1, 1], F32, tag="tsb")
        nc.vector.tensor_copy(out=tsb, in_=tps)
        res = spool.tile([1, 1], F32, tag="res")
        nc.vector.tensor_sub(res, tsb, arsb[0:1, 512:513])
        nc.sync.dma_start(out=out_ap, in_=res)


_NC_CACHE = {}


def build_nc():
    key = ("nc", HOST_COMBINE)
    if key in _NC_CACHE:
        return _NC_CACHE[key]
    nc = bacc.Bacc(
        "TRN2", target_bir_lowering=False, debug=False, num_devices=NCORES
    )
    cit8 = nc.dram_tensor("cit8", [ICH, 128, KT, 512], F8, kind="ExternalInput").ap()
    cj16 = nc.dram_tensor("cj16", [SHARD, D], F16, kind="ExternalInput").ap()
    cjt16 = nc.dram_tensor("cjt16", [JT, 128, KT, 128], F16, kind="ExternalInput").ap()
    cid16 = nc.dram_tensor("cid16", [SHARD, D], F16, kind="ExternalInput").ap()
    oh = nc.dram_tensor("oh", [128, 8, 8], BF16, kind="ExternalInput").ap()
    out_shape = [8, 513] if HOST_COMBINE else [1, 1]
    out = nc.dram_tensor("out", out_shape, F32, kind="ExternalOutput").ap()
    dbg = None
    if DEBUG_OUT:
        dbg = {
            "zab": nc.dram_tensor("d_zab", [128, 5 * JT], F32, kind="ExternalOutput").ap(),
            "texp": nc.dram_tensor("d_texp", [128, B], F32, kind="ExternalOutput").ap(),
            "cssb": nc.dram_tensor("d_cssb", [8, 513], F32, kind="ExternalOutput").ap(),
            "arsb": nc.dram_tensor("d_arsb", [8, 513], F32, kind="ExternalOutput").ap(),
        }
    with tile.TileContext(nc) as tc:
        build_kernel_body(tc, out, cit8, cj16, cjt16, cid16, oh, dbg=dbg,
                          host_combine=HOST_COMBINE)
    nc.compile()
    _NC_CACHE[key] = nc
    return nc


def make_in_maps(c_i, c_j):
    c_i = np.ascontiguousarray(np.asarray(c_i, dtype=np.float32))
    c_j = np.ascontiguousarray(np.asarray(c_j, dtype=np.float32))
    ci8 = c_i.astype(ml_dtypes.float8_e4m3)
    # cit8[c, p, kt, i'] = c_i[512c + i', kt*128 + p]  (chunk-major, so each
    # chunk DMA is one fully contiguous transfer)
    cit8 = np.ascontiguousarray(
        ci8.T.reshape(KT, 128, ICH, 512).transpose(2, 1, 0, 3)
    )
    ci16 = c_i.astype(np.float16)
    cj16 = c_j.astype(np.float16)
    oh = np.zeros((128, 8, 8), ml_dtypes.bfloat16)
    for r in range(8):
        oh[:, r, r] = 1.0
    in_maps = []
    for c in range(NCORES):
        cjs = cj16[SHARD * c : SHARD * (c + 1)]
        # cjt16[jt, p, kt, j'] = c_j[512c + 128*jt + j', kt*128 + p]
        cjt = np.ascontiguousarray(
            cjs.T.reshape(KT, 128, JT, 128).transpose(2, 1, 0, 3)
        )
        in_maps.append(
            {
                "cit8": cit8,
                "cj16": np.ascontiguousarray(cjs),
                "cjt16": cjt,
                "cid16": np.ascontiguousarray(ci16[SHARD * c : SHARD * (c + 1)]),
                "oh": oh,
            }
        )
    return in_maps


def kernel(c_i, c_j, **kwargs):
    nc = build_nc()
    in_maps = make_in_maps(c_i, c_j)
    res = bass_utils.run_bass_kernel_spmd(
        nc, in_maps, core_ids=list(range(NCORES))
    )
    if HOST_COMBINE:
        acc = np.zeros((8, 513), np.float64)
        for r in res.results:
            acc += r["out"].astype(np.float64)
        lossB = np.log(acc[:, 0:512]).sum() - acc[0, 512]
        return np.float32(lossB / B).reshape(())
    return np.float32(np.float64(res.results[0]["out"][0, 0]) / B).reshape(())
